# revision 20
# baseline (speedup 1.0000x reference)
"""Multi-head attention (B=16, C=256, N=1024, H=4 heads) on 8 TRN2 NeuronCores.

Data-parallel over batch: 2 images per core, weights replicated, no
collectives.

v2: every GEMM runs in fp8e4m3 with DoubleRow perf mode. Trace analysis of
the bf16 version showed MATMUL issue-to-issue spacing is ~259ns for N=512
regardless of dtype, and fp8 DR contracts 256 rows per slot vs 128 for
bf16 — exactly 2x FLOPs per slot (LDWEIGHTS ~162ns rides fully hidden).
This halves PE busy time from ~220us to ~116us per core. Simulated
end-to-end rel err ~8e-3 (gate 2e-2).

Layouts are all "transposed" ([feature, token]) as before — zero on-chip
transposes. fp8 packing for DoubleRow: both operands are [128, 2, X] APs
where the contraction index is (partition + 128*plane):
  qkT[4mt, N]   = W_qk.T @ x          (lhsT = W k-planes, rhs = xb k-planes)
  attT[j, i]    = k @ q.T             (lhsT = kT d-planes, rhs = qT d-planes)
  E'            = exp(attT/16)*2^-6   (ScalarE, PSUM -> SBUF fp8; 2^-6 keeps
                                       maxE ~48 << 240 fp8e4 sat limit)
  outT[d, i]    = v.T @ E'            (lhsT = v jt-planes)
  s[i]          = ones.T @ E'         (128-row broadcast denominator)
  resT[c, i]    = W_out.T @ catT + x_r + bias
Softmax normalization divides E'-scaled numerator by E'-scaled s: exact.

Engine budget per image (measured cost model): PE 224 MM slots ~58us,
ScalarE (all exp drains + q/xrb) ~55us, DVE (k/v drains, normalize,
recip, residual, casts) ~50us. PSUM: shared 2-bank work pool bufs=3
(6 banks) + AV accumulator (2 banks). Drains move >=1024 elem per op.
Emission is software-pipelined: scores(h+1) interleave with AV(h) so the
ScalarE exp-drain latency hides under AV/qk matmuls.
"""
import sys

try:
    import concourse.bass as bass  # noqa: F401
except ImportError:
    sys.path.insert(0, "/opt/trn_rl_repo")

from contextlib import ExitStack

import numpy as np

import concourse.bass as bass
import concourse.mybir as mybir
import concourse.tile as tile
from concourse import bacc
from concourse.bass_utils import run_bass_kernel_spmd

F32 = mybir.dt.float32
FP8 = mybir.dt.float8e4
EXP = mybir.ActivationFunctionType.Exp
IDENT = mybir.ActivationFunctionType.Identity
DR = mybir.MatmulPerfMode.DoubleRow
MUL = mybir.AluOpType.mult

B_PER_CORE = 2   # 16 images / 8 cores
C = 256          # channels == head dim
N = 1024         # tokens (32*32)
HEADS = 4
SCALE = C ** -0.5
E_BIAS = float(np.log(2.0 ** -6))  # exp pre-scale: E' = exp(s/16)*2^-6
N_CORES = 8


def _build():
    nc = bacc.Bacc("TRN2", debug=False, num_devices=N_CORES)
    x_d = nc.declare_dram_parameter("x", [B_PER_CORE, C, N], F32, isOutput=False)
    wp_d = nc.declare_dram_parameter("W_proj", [C, 3 * HEADS * C], F32, isOutput=False)
    bp_d = nc.declare_dram_parameter("b_proj", [3 * HEADS * C], F32, isOutput=False)
    wo_d = nc.declare_dram_parameter("W_out", [HEADS * C, C], F32, isOutput=False)
    bo_d = nc.declare_dram_parameter("b_out", [C], F32, isOutput=False)
    out_d = nc.declare_dram_parameter("out", [B_PER_CORE, C, N], F32, isOutput=True)

    with tile.TileContext(nc) as tc, ExitStack() as ctx:
        pool = ctx.enter_context(tc.tile_pool(name="persist", bufs=1))
        stage_pool = ctx.enter_context(tc.tile_pool(name="stage", bufs=3))
        xr_pool = ctx.enter_context(tc.tile_pool(name="xr", bufs=2))
        xb_pool = ctx.enter_context(tc.tile_pool(name="xb", bufs=2))
        v2_pool = ctx.enter_context(tc.tile_pool(name="v2", bufs=2))
        qk_pool = ctx.enter_context(tc.tile_pool(name="qk", bufs=2))
        e_pool = ctx.enter_context(tc.tile_pool(name="e", bufs=2))
        cat_pool = ctx.enter_context(tc.tile_pool(name="cat", bufs=2))
        r_pool = ctx.enter_context(tc.tile_pool(name="r", bufs=2))
        xrb_pool = ctx.enter_context(tc.tile_pool(name="xrb", bufs=2))
        out_pool = ctx.enter_context(tc.tile_pool(name="outs", bufs=2))
        ps_work = ctx.enter_context(tc.tile_pool(name="psw", bufs=3, space="PSUM"))
        ps_acc = ctx.enter_context(tc.tile_pool(name="psa", bufs=1, space="PSUM"))

        # ---- constants first: DVE memsets precede everything in its FIFO ----
        ones8 = pool.tile([128, 2, 128], FP8)
        nc.vector.memset(ones8[:], 1.0)
        eb_sb = pool.tile([128, 1], F32)  # exp bias: ln(2^-6)
        nc.vector.memset(eb_sb[:], E_BIAS)
        wrm = pool.tile([128, 2, 512], FP8)
        nc.vector.memset(wrm[:], 1.0)

        # sacrificial exp: forces the ~2.7us ACT_TABLE_LOAD during the DMA
        # wait instead of stalling the first qk drain (which re-throttled HAM)
        scr = pool.tile([128, 1], F32)
        nc.scalar.activation(scr[:], eb_sb[:], EXP)

        # dummy DR matmuls: fill the initial DMA wait + warm the HAM clock gate
        for wi in range(19):
            warm_ps = ps_work.tile([128, 2, 512], F32, tag="work")
            nc.tensor.matmul(out=warm_ps[:, wi % 2, :], lhsT=ones8[:],
                             rhs=wrm[:], start=True, stop=True, perf_mode=DR)

        # ---- input DMAs + fp8 casts, first-needed data first ----
        xr_tiles = []
        xr = xr_pool.tile([128, 2, N], F32, tag="xr")
        for kt in range(2):
            for isl in range(2):
                nc.sync.dma_start(
                    out=xr[:, kt, isl * 512:(isl + 1) * 512],
                    in_=x_d[0, kt * 128:(kt + 1) * 128, isl * 512:(isl + 1) * 512])
        xr_tiles.append(xr)

        # image 0's fp8 x on DVE; weight casts ride idle GPSIMD so they never
        # block the DVE FIFO (k-drains/v-drains) at image-0 start
        xb0 = xb_pool.tile([128, 2, N], FP8, tag="xb")
        for kt in range(2):
            nc.vector.tensor_copy(xb0[:, kt, :], xr[:, kt, :])

        # W_proj: q,k cols (first 512 of each 768 block) -> wqk; v cols -> wv
        wqk = pool.tile([128, 2, 4 * 512], FP8)
        wv = pool.tile([128, 2, 4 * 256], FP8)
        b_sb = None
        for h in range(HEADS):
            for kt in range(2):
                ws = stage_pool.tile([128, 512], F32, tag="wstage")
                nc.sync.dma_start(
                    out=ws[:],
                    in_=wp_d[kt * 128:(kt + 1) * 128, h * 768:h * 768 + 512])
                # h0 cast on DVE (fast, first-needed); rest on idle GPSIMD
                eng = nc.vector if h == 0 else nc.gpsimd
                eng.tensor_copy(wqk[:, kt, h * 512:(h + 1) * 512], ws[:])
            if h == 0:
                # v weights for all heads (needed right after head-0 qk)
                for kt in range(2):
                    vs = stage_pool.tile([128, 4, 256], F32, tag="vstage")
                    nc.sync.dma_start(
                        out=vs[:],
                        in_=wp_d[kt * 128:(kt + 1) * 128, :].rearrange(
                            "p (h x) -> p h x", h=4)[:, :, 512:768])
                    nc.vector.tensor_copy(
                        wv[:, kt, :].rearrange("p (h x) -> p h x", h=4), vs[:])
                b_sb = pool.tile([128, 24], F32)  # b_proj, tile t
                nc.sync.dma_start(
                    out=b_sb[:], in_=bp_d[:].rearrange("(t p) -> p t", p=128))
                bo_sb = pool.tile([128, 2], F32)
                nc.sync.dma_start(out=bo_sb[:],
                                  in_=bo_d[:].rearrange("(t p) -> p t", p=128))

        # second image's x: queued last, prefetched during image-0 compute
        xr = xr_pool.tile([128, 2, N], F32, tag="xr")
        for kt in range(2):
            nc.sync.dma_start(out=xr[:, kt, :],
                              in_=x_d[1, kt * 128:(kt + 1) * 128, :])
        xr_tiles.append(xr)

        total_bias = pool.tile([128, 2], F32)
        wo_sb = pool.tile([128, 8, 256], FP8)  # W_out kt-tiles (loaded early img 0)
        zb = pool.tile([128, 8, 2], FP8)

        def emit_qk(h, qk_t, xb):
            """q,k for head h -> qk_t[128, 4(q0 q1 k0 k1), N] fp8."""
            for mt in range(4):
                ps = ps_work.tile([128, 2, 512], F32, tag="work")
                lhs = wqk[:, :, h * 512 + mt * 128:h * 512 + (mt + 1) * 128]
                for isl in range(2):
                    nc.tensor.matmul(
                        out=ps[:, isl, :], lhsT=lhs,
                        rhs=xb[:, :, isl * 512:(isl + 1) * 512],
                        start=True, stop=True, perf_mode=DR)
                col = h * 6 + mt
                dest = qk_t[:, mt, :].rearrange("p (a x) -> p a x", a=2)
                if mt < 2:  # q rows: ScalarE
                    nc.scalar.activation(dest, ps[:], IDENT,
                                         bias=b_sb[:, col:col + 1])
                else:       # k rows: DVE
                    nc.vector.tensor_scalar_add(dest, ps[:], b_sb[:, col:col + 1])

        def emit_v(hp, v2, xb):
            """v for heads 2hp, 2hp+1 -> v2[:, it, h*256+d] (natural layout)."""
            for itp in range(4):
                ps = ps_work.tile([128, 2, 512], F32, tag="work")
                for j in range(2):
                    it = 2 * itp + j
                    nc.tensor.matmul(
                        out=ps[:, j, :],
                        lhsT=xb[:, :, it * 128:(it + 1) * 128],
                        rhs=wv[:, :, hp * 512:(hp + 1) * 512],
                        start=True, stop=True, perf_mode=DR)
                dest = v2[:, 2 * itp:2 * itp + 2, hp * 512:(hp + 1) * 512]
                nc.vector.tensor_copy(dest, ps[:])

        def emit_scores(h, qk_t, e_t, isl):
            """attT jt-pair tiles -> E' = exp(attT/16)*2^-6 in fp8."""
            for a in range(4):
                ps = ps_work.tile([128, 2, 512], F32, tag="work")
                for j in range(2):
                    jt = 2 * a + j
                    nc.tensor.matmul(
                        out=ps[:, j, :],
                        lhsT=qk_t[:, 2:4, jt * 128:(jt + 1) * 128],
                        rhs=qk_t[:, 0:2, isl * 512:(isl + 1) * 512],
                        start=True, stop=True, perf_mode=DR)
                nc.scalar.activation(
                    e_t[:, 2 * a:2 * a + 2, isl * 512:(isl + 1) * 512],
                    ps[:], EXP, scale=SCALE, bias=eb_sb[:])

        def emit_av(h, e_t, v2, cat, isl):
            """denominator + AV for one i-half; normalized into catT (fp8).
            Interleaved per jt-pair so consumption tracks the ScalarE
            exp-drain arrival instead of bursting ahead of it."""
            s_t = ps_work.tile([128, 2, 512], F32, tag="work")
            s_ps = s_t[:, 0, :]
            o_ps = ps_acc.tile([128, 2, 512], F32, tag="acc")
            for a in range(4):
                e_ap = e_t[:, 2 * a:2 * a + 2, isl * 512:(isl + 1) * 512]
                nc.tensor.matmul(
                    out=s_ps, lhsT=ones8[:], rhs=e_ap,
                    perf_mode=DR, start=(a == 0), stop=(a == 3))
                for dh in range(2):
                    nc.tensor.matmul(
                        out=o_ps[:, dh, :],
                        lhsT=v2[:, 2 * a:2 * a + 2,
                                h * 256 + dh * 128:h * 256 + (dh + 1) * 128],
                        rhs=e_ap, start=(a == 0), stop=(a == 3), perf_mode=DR)
            r_sb = r_pool.tile([128, 512], F32, tag="r")
            nc.vector.reciprocal_approx_fast(r_sb[:], s_ps)
            for dh in range(2):
                nc.vector.scalar_tensor_tensor(
                    cat[:, 2 * h + dh, isl * 512:(isl + 1) * 512],
                    o_ps[:, dh, :], 1.0, r_sb[:], MUL, MUL)

        for b in range(B_PER_CORE):
            xr = xr_tiles[b]
            if b == 0:
                xb = xb0
            else:
                xb = xb_pool.tile([128, 2, N], FP8, tag="xb")
                for kt in range(2):
                    nc.vector.tensor_copy(xb[:, kt, :], xr[:, kt, :])
            v2 = v2_pool.tile([128, 8, 1024], FP8, tag="v2")
            cat = cat_pool.tile([128, 8, N], FP8, tag="cat")

            qk_t = {0: qk_pool.tile([128, 4, N], FP8, tag="qk", name="qk_t")}
            emit_qk(0, qk_t[0], xb)
            emit_v(0, v2, xb)
            e_tt = {0: e_pool.tile([128, 8, N], FP8, tag="e", name="e_t")}
            emit_scores(0, qk_t[0], e_tt[0], 0)
            emit_scores(0, qk_t[0], e_tt[0], 1)

            for h in range(HEADS):
                if h + 1 < HEADS:
                    qk_t[h + 1] = qk_pool.tile([128, 4, N], FP8, tag="qk",
                                               name="qk_t")
                    emit_qk(h + 1, qk_t[h + 1], xb)
                if h == 1:
                    emit_v(1, v2, xb)
                if b == 0 and h == 0:
                    # W_out + deferred-bias setup on idle GPSIMD
                    wos = stage_pool.tile([128, 8, 256], F32, tag="wostage")
                    nc.sync.dma_start(
                        out=wos[:],
                        in_=wo_d[:, :].rearrange("(t p) c -> p t c", p=128))
                    nc.gpsimd.tensor_copy(wo_sb[:], wos[:])
                    zscr = stage_pool.tile([128, 16], F32, tag="zscr")
                    nc.gpsimd.memset(zscr[:], 0.0)
                    nc.gpsimd.tensor_copy(
                        zb[:], zscr[:].rearrange("p (a c) -> p a c", c=2))
                    for kt in range(8):
                        hh, dt = kt // 2, kt % 2
                        nc.gpsimd.tensor_copy(
                            zb[:, kt, 0:1],
                            b_sb[:, hh * 6 + 4 + dt:hh * 6 + 5 + dt])
                emit_av(h, e_tt[h], v2, cat, 0)
                if h + 1 < HEADS:
                    e_tt[h + 1] = e_pool.tile([128, 8, N], FP8, tag="e",
                                              name="e_t")
                    emit_scores(h + 1, qk_t[h + 1], e_tt[h + 1], 0)
                emit_av(h, e_tt[h], v2, cat, 1)
                if h + 1 < HEADS:
                    emit_scores(h + 1, qk_t[h + 1], e_tt[h + 1], 1)

            if b == 0:
                # b_v folds through softmax (weights sum to 1) and W_out:
                # total_bias[c] = b_out[c] + sum_hd b_v[hd] * W_out[hd, c].
                for ct in range(2):
                    bias_ps = ps_work.tile([128, 2], F32, tag="work")
                    for kt in range(8):
                        nc.tensor.matmul(out=bias_ps[:],
                                         lhsT=wo_sb[:, kt, ct * 128:(ct + 1) * 128],
                                         rhs=zb[:, kt, :],
                                         start=(kt == 0), stop=(kt == 7))
                    nc.vector.tensor_add(total_bias[:, ct:ct + 1], bias_ps[:, 0:1],
                                         bo_sb[:, ct:ct + 1])

            # residual + bias, broadcast along tokens: xrb = x_r + total_bias
            xrb = xrb_pool.tile([128, 2, 2, 512], F32, tag="xrb")
            for ct in range(2):
                nc.vector.tensor_scalar_add(
                    xrb[:, ct],
                    xr[:, ct, :].rearrange("p (a x) -> p a x", a=2),
                    total_bias[:, ct:ct + 1])

            # ---- out projection + residual, already in output layout ----
            for ct in range(2):
                res_ps = ps_work.tile([128, 2, 512], F32, tag="work")
                for isl in range(2):
                    for t in range(4):
                        nc.tensor.matmul(
                            out=res_ps[:, isl, :],
                            lhsT=wo_sb[:, 2 * t:2 * t + 2, ct * 128:(ct + 1) * 128],
                            rhs=cat[:, 2 * t:2 * t + 2, isl * 512:(isl + 1) * 512],
                            start=(t == 0), stop=(t == 3), perf_mode=DR)
                o_sb = out_pool.tile([128, 2, 512], F32, tag="o_sb")
                for isl in range(2):  # per-half so the store starts earlier
                    nc.vector.tensor_add(o_sb[:, isl, :], res_ps[:, isl, :],
                                         xrb[:, ct, isl])
                    nc.sync.dma_start(
                        out=out_d[b, ct * 128:(ct + 1) * 128,
                                  isl * 512:(isl + 1) * 512],
                        in_=o_sb[:, isl, :])

    nc.compile()
    return nc


_NC = None


def kernel(x, W_proj, b_proj, W_out, b_out):
    global _NC
    if _NC is None:
        _NC = _build()
    x = np.ascontiguousarray(x, dtype=np.float32).reshape(16, C, N)
    in_maps = [
        {
            "x": x[i * B_PER_CORE:(i + 1) * B_PER_CORE],
            "W_proj": np.ascontiguousarray(W_proj, dtype=np.float32),
            "b_proj": np.ascontiguousarray(b_proj, dtype=np.float32),
            "W_out": np.ascontiguousarray(W_out, dtype=np.float32),
            "b_out": np.ascontiguousarray(b_out, dtype=np.float32),
        }
        for i in range(N_CORES)
    ]
    res = run_bass_kernel_spmd(_NC, in_maps, core_ids=list(range(N_CORES)))
    out = np.concatenate([res.results[i]["out"] for i in range(N_CORES)], axis=0)
    return out.reshape(16, C, 32, 32)
